# revision 6
# baseline (speedup 1.0000x reference)
"""Bass/Trainium2 kernel for the attention-LSTM decoder (nn_Decoder).

Strategy:
  - The 2-layer LSTM + dot-product attention recurrence (T=64 steps) is
    inherently sequential; it is replicated on all 8 cores (batch 32 each).
    Per-step state lives in SBUF in a transposed layout: feature dims on
    partitions, batch (32) on the free axis, so every matmul is
    weights-stationary [K=128, M=128] x moving [K=128, N=32].
  - Attention scores are computed as a cross-batch outer product
    (h1T stationary, encT moving) -> PSUM [32, (b,s)=2048]; the per-batch
    diagonal blocks are extracted with a flat-DRAM round trip (arbitrary
    strided APs are legal on the DRAM side of a DMA).
  - ctx = attn @ enc[b] is computed with a block-diagonal moving operand:
    enc_flat [(b,s),h] chunks stationary, attn scattered block-diagonally
    into [(b,s), 32] (only col b nonzero for rows of batch b).
  - The vocab projection (dominant FLOPs/memory) is hoisted out of the time
    loop entirely: concat_t = [h1_t, ctx_t, emb_t] is staged to DRAM each
    step; afterwards one big (2048 x 1536) @ (1536 x 4000) matmul per core,
    vocab-sharded 8 ways.
  - Only the `exp` ACT table set is used: sigmoid(x) = 0.5*tanh(x/2)+0.5.
  - All matmuls run in bf16 with fp32 PSUM accumulation; LSTM state is fp32.
"""

import sys

if "/opt/trn_rl_repo" not in sys.path:
    sys.path.insert(0, "/opt/trn_rl_repo")

import numpy as np
import ml_dtypes

B, T, S = 32, 64, 64
V, E, H = 32000, 512, 512
NCORES = 8
VS = V // NCORES          # 4000 vocab rows per core
BT = B * T                # 2048
NF = 500                  # fc psum free-dim chunk (8 chunks of 500 = 4000)
INV_SQRT_H = 1.0 / float(np.sqrt(H))

_CACHE = {}


def _build_program(t_steps):
    import concourse.bass as bass
    import concourse.mybir as mybir
    import concourse.tile as tile
    from concourse import bacc

    f32 = mybir.dt.float32
    bf16 = mybir.dt.bfloat16
    i32 = mybir.dt.int32
    Tanh = mybir.ActivationFunctionType.Tanh
    Exp = mybir.ActivationFunctionType.Exp
    X = mybir.AxisListType.X

    nc = bacc.Bacc("TRN2", target_bir_lowering=False, debug=False,
                   num_devices=NCORES)

    # ---- kernel I/O ----
    idx_d = nc.declare_dram_parameter("idx", [16, 128], i32, isOutput=False)
    emb_d = nc.declare_dram_parameter("embt", [V, E], bf16, isOutput=False)
    encT_d = nc.declare_dram_parameter("encT", [4, 128, BT], bf16, isOutput=False)
    encF_d = nc.declare_dram_parameter("encF", [16, 128, H], bf16, isOutput=False)
    w0_d = nc.declare_dram_parameter("w0t", [12, 128, 2048], bf16, isOutput=False)
    w1_d = nc.declare_dram_parameter("w1t", [8, 128, 2048], bf16, isOutput=False)
    fc_d = nc.declare_dram_parameter("fct", [12, 128, VS], bf16, isOutput=False)
    h0_d = nc.declare_dram_parameter("h0t", [2, 4, 128, 32], f32, isOutput=False)
    c0_d = nc.declare_dram_parameter("c0t", [2, 4, 128, 32], f32, isOutput=False)

    logits_d = nc.declare_dram_parameter("logits", [BT, VS], f32, isOutput=True)
    attn_d = nc.declare_dram_parameter("attn_o", [T, 32, 64], f32, isOutput=True)
    h_d = nc.declare_dram_parameter("h_o", [2, 4, 128, 32], f32, isOutput=True)
    c_d = nc.declare_dram_parameter("c_o", [2, 4, 128, 32], f32, isOutput=True)

    # ---- internal DRAM ----
    scd = nc.dram_tensor("scd", [32, 2048], f32)
    cct = nc.dram_tensor("cct", [12, 128, BT], bf16)
    abd2_d = nc.dram_tensor("abd2", [128, 16, 32], bf16)

    def dram_ap(tensor, offset, pattern):
        return bass.AP(tensor=tensor, offset=offset, ap=pattern)

    with tile.TileContext(nc) as tc:
        with (
            tc.tile_pool(name="per", bufs=1) as per,
            tc.tile_pool(name="work", bufs=2) as work,
            tc.tile_pool(name="cstp", bufs=2) as cstp,
        ):
            # ======== prologue: load everything resident ========
            idx_sb = per.tile([128, 16], i32)
            nc.sync.dma_start(out=idx_sb[:], in_=idx_d.rearrange("c p -> p c"))

            embcat = per.tile([128, 16, E], bf16)
            for c in range(16):
                nc.gpsimd.indirect_dma_start(
                    out=embcat[:, c, :],
                    out_offset=None,
                    in_=emb_d[:],
                    in_offset=bass.IndirectOffsetOnAxis(ap=idx_sb[:, c:c + 1], axis=0),
                )

            encT_sb = per.tile([128, 4, BT], bf16)
            nc.sync.dma_start(out=encT_sb[:], in_=encT_d.rearrange("k p n -> p k n"))
            encF_sb = per.tile([128, 16, H], bf16)
            nc.sync.dma_start(out=encF_sb[:], in_=encF_d.rearrange("c p h -> p c h"))
            w0_sb = per.tile([128, 12, 2048], bf16)
            nc.sync.dma_start(out=w0_sb[:], in_=w0_d.rearrange("k p m -> p k m"))
            w1_sb = per.tile([128, 8, 2048], bf16)
            nc.sync.dma_start(out=w1_sb[:], in_=w1_d.rearrange("k p m -> p k m"))

            # persistent state (feature-on-partition, batch-on-free layout)
            h0f = per.tile([128, 4, 32], f32)
            h1f = per.tile([128, 4, 32], f32)
            c0f = per.tile([128, 4, 32], f32)
            c1f = per.tile([128, 4, 32], f32)
            h0bf = per.tile([128, 4, 32], bf16)
            h1bf = per.tile([128, 4, 32], bf16)
            nc.sync.dma_start(out=h0f[:], in_=h0_d[0].rearrange("k p b -> p k b"))
            nc.sync.dma_start(out=h1f[:], in_=h0_d[1].rearrange("k p b -> p k b"))
            nc.sync.dma_start(out=c0f[:], in_=c0_d[0].rearrange("k p b -> p k b"))
            nc.sync.dma_start(out=c1f[:], in_=c0_d[1].rearrange("k p b -> p k b"))
            nc.vector.tensor_copy(h0bf[:], h0f[:])
            nc.vector.tensor_copy(h1bf[:], h1f[:])

            abd_sb = per.tile([128, 16, 32], bf16)  # block-diagonal attn
            nc.vector.memset(abd_sb[:], 0.0)
            nc.sync.dma_start(out=abd2_d[:], in_=abd_sb[:])  # zero the image

            def lstm_nonlin(gps, cstate, hf_out, hbf_out, extra_bf_out):
                """gps: [128,16,32] psum of gates (i,f,g,o x 4 tiles each)."""
                t_i = work.tile([128, 4, 32], f32, tag="tgi")
                t_f = work.tile([128, 4, 32], f32, tag="tgf")
                t_g = work.tile([128, 4, 32], f32, tag="tgg")
                t_o = work.tile([128, 4, 32], f32, tag="tgo")
                nc.scalar.activation(t_i[:], gps[:, 0:4, :], Tanh, scale=0.5)
                nc.scalar.activation(t_f[:], gps[:, 4:8, :], Tanh, scale=0.5)
                nc.scalar.activation(t_g[:], gps[:, 8:12, :], Tanh)
                nc.scalar.activation(t_o[:], gps[:, 12:16, :], Tanh, scale=0.5)
                si = work.tile([128, 4, 32], f32, tag="si")
                sf = work.tile([128, 4, 32], f32, tag="sf")
                so = work.tile([128, 4, 32], f32, tag="so")
                nc.vector.tensor_scalar(si[:], t_i[:], 0.5, 0.5,
                                        op0=mybir.AluOpType.mult,
                                        op1=mybir.AluOpType.add)
                nc.vector.tensor_scalar(sf[:], t_f[:], 0.5, 0.5,
                                        op0=mybir.AluOpType.mult,
                                        op1=mybir.AluOpType.add)
                nc.vector.tensor_scalar(so[:], t_o[:], 0.5, 0.5,
                                        op0=mybir.AluOpType.mult,
                                        op1=mybir.AluOpType.add)
                fc_ = work.tile([128, 4, 32], f32, tag="fc_")
                ig = work.tile([128, 4, 32], f32, tag="ig")
                nc.vector.tensor_mul(fc_[:], sf[:], cstate[:])
                nc.vector.tensor_mul(ig[:], si[:], t_g[:])
                nc.vector.tensor_add(cstate[:], fc_[:], ig[:])
                t_c = work.tile([128, 4, 32], f32, tag="tc")
                nc.scalar.activation(t_c[:], cstate[:], Tanh)
                nc.vector.tensor_mul(hf_out[:], so[:], t_c[:])
                nc.vector.tensor_copy(hbf_out[:], hf_out[:])
                if extra_bf_out is not None:
                    nc.vector.tensor_copy(extra_bf_out, hbf_out[:])

            with (
                tc.tile_pool(name="scps", bufs=1, space="PSUM") as scps,
                tc.tile_pool(name="ctxps", bufs=1, space="PSUM") as ctxps,
                tc.tile_pool(name="g0ps", bufs=1, space="PSUM") as g0psp,
                tc.tile_pool(name="g1ps", bufs=1, space="PSUM") as g1psp,
            ):
                for t in range(t_steps):
                    # ---- attention scores: cross product [32b x (b,s)] ----
                    scp = scps.tile([32, 2048], f32, tag="scp")
                    for nb in range(4):
                        for k in range(4):
                            nc.tensor.matmul(
                                scp[:, nb * 512:(nb + 1) * 512],
                                h1bf[:, k, :],
                                encT_sb[:, k, nb * 512:(nb + 1) * 512],
                                start=(k == 0), stop=(k == 3),
                            )
                    scsb = work.tile([32, 2048], f32, tag="scsb")
                    nc.vector.tensor_copy(scsb[:], scp[:])
                    nc.sync.dma_start(out=scd[:], in_=scsb[:])
                    dg = work.tile([32, 64], f32, tag="dg")
                    nc.sync.dma_start(
                        out=dg[:],
                        in_=dram_ap(scd, 0, [[2112, 32], [1, 64]]),
                    )
                    # ---- softmax over s (scale by 1/sqrt(H) inside exp) ----
                    mx = work.tile([32, 1], f32, tag="mx")
                    nc.vector.reduce_max(mx[:], dg[:], axis=X)
                    mxn = work.tile([32, 1], f32, tag="mxn")
                    nc.vector.tensor_scalar_mul(mxn[:], mx[:], -INV_SQRT_H)
                    ex = work.tile([32, 64], f32, tag="ex")
                    se = work.tile([32, 1], f32, tag="se")
                    nc.scalar.activation(ex[:], dg[:], Exp,
                                         bias=mxn[:, :1], scale=INV_SQRT_H,
                                         accum_out=se[:, :1])
                    rse = work.tile([32, 1], f32, tag="rse")
                    nc.vector.reciprocal(rse[:], se[:])
                    attn_f = work.tile([32, 64], f32, tag="attn_f")
                    nc.vector.tensor_scalar_mul(attn_f[:], ex[:], rse[:, :1])
                    nc.sync.dma_start(out=attn_d[t], in_=attn_f[:])
                    attn_b = work.tile([32, 64], bf16, tag="attn_b")
                    nc.vector.tensor_copy(attn_b[:], attn_f[:])
                    # scatter diag into DRAM block-diag image, then load
                    for bh in range(2):
                        nc.sync.dma_start(
                            out=dram_ap(abd2_d, bh * 64 * 512 + bh,
                                        [[34, 16], [512, 64]]),
                            in_=attn_b[bh::2, :],
                        )
                    nc.sync.dma_start(out=abd_sb[:], in_=abd2_d[:])
                    # ---- ctx: block-diag moving operand ----
                    ctp = ctxps.tile([128, 4, 32], f32, tag="ctp")
                    for hk in range(4):
                        for c in range(16):
                            nc.tensor.matmul(
                                ctp[:, hk, :],
                                encF_sb[:, c, hk * 128:(hk + 1) * 128],
                                abd_sb[:, c, :],
                                start=(c == 0), stop=(c == 15),
                            )
                    # ---- concat staging tile: [h1(0:4) ctx(4:8) emb(8:12)] ----
                    cst = cstp.tile([128, 12, 32], bf16, tag="cst")
                    nc.vector.tensor_copy(cst[:, 4:8, :], ctp[:])
                    # embT via 16 DVE 32x32 block transposes
                    c_t, r_t = t // 4, (t % 4) * 32
                    for hk in range(4):
                        for j in range(4):
                            nc.vector.transpose(
                                cst[32 * j:32 * (j + 1), 8 + hk, :],
                                embcat[r_t:r_t + 32, c_t,
                                       hk * 128 + 32 * j: hk * 128 + 32 * (j + 1)],
                            )
                    # ---- layer-0 gates ----
                    g0 = g0psp.tile([128, 16, 32], f32, tag="g0")
                    rhs0 = ([cst[:, 8 + k, :] for k in range(4)]
                            + [cst[:, 4 + k, :] for k in range(4)]
                            + [h0bf[:, k, :] for k in range(4)])
                    for m in range(16):
                        for kt in range(12):
                            nc.tensor.matmul(
                                g0[:, m, :],
                                w0_sb[:, kt, m * 128:(m + 1) * 128],
                                rhs0[kt],
                                start=(kt == 0), stop=(kt == 11),
                            )
                    lstm_nonlin(g0, c0f, h0f, h0bf, None)
                    # ---- layer-1 gates ----
                    g1 = g1psp.tile([128, 16, 32], f32, tag="g1")
                    rhs1 = ([h0bf[:, k, :] for k in range(4)]
                            + [h1bf[:, k, :] for k in range(4)])
                    for m in range(16):
                        for kt in range(8):
                            nc.tensor.matmul(
                                g1[:, m, :],
                                w1_sb[:, kt, m * 128:(m + 1) * 128],
                                rhs1[kt],
                                start=(kt == 0), stop=(kt == 7),
                            )
                    lstm_nonlin(g1, c1f, h1f, h1bf, cst[:, 0:4, :])
                    # ---- stage concat column block to DRAM ----
                    nc.sync.dma_start(
                        out=dram_ap(cct, t * 32,
                                    [[BT, 128], [128 * BT, 12], [1, 32]]),
                        in_=cst[:],
                    )

            # ---- final states out ----
            nc.sync.dma_start(out=h_d[0].rearrange("k p b -> p k b"), in_=h0f[:])
            nc.sync.dma_start(out=h_d[1].rearrange("k p b -> p k b"), in_=h1f[:])
            nc.sync.dma_start(out=c_d[0].rearrange("k p b -> p k b"), in_=c0f[:])
            nc.sync.dma_start(out=c_d[1].rearrange("k p b -> p k b"), in_=c1f[:])

            # ======== fc epilogue: logits = concat @ fc_W.T ========
            with (
                tc.tile_pool(name="fcw", bufs=2) as fcwp,
                tc.tile_pool(name="lhs", bufs=3) as lhsp,
                tc.tile_pool(name="fps", bufs=2, space="PSUM") as fpsp,
                tc.tile_pool(name="lg", bufs=3) as lgp,
            ):
                for n in range(VS // NF):
                    fcw = fcwp.tile([128, 12, NF], bf16, tag="fcw")
                    nc.sync.dma_start(
                        out=fcw[:],
                        in_=dram_ap(fc_d, n * NF,
                                    [[VS, 128], [128 * VS, 12], [1, NF]]),
                    )
                    for m in range(16):
                        lhs = lhsp.tile([128, 12, 128], bf16, tag="lhs")
                        nc.sync.dma_start(
                            out=lhs[:],
                            in_=dram_ap(cct, m * 128,
                                        [[BT, 128], [128 * BT, 12], [1, 128]]),
                        )
                        fps = fpsp.tile([128, NF], f32, tag="fps")
                        for kt in range(12):
                            nc.tensor.matmul(
                                fps[:], lhs[:, kt, :], fcw[:, kt, :],
                                start=(kt == 0), stop=(kt == 11),
                            )
                        lg = lgp.tile([128, NF], f32, tag="lg")
                        nc.vector.tensor_copy(lg[:], fps[:])
                        nc.sync.dma_start(
                            out=dram_ap(logits_d, m * 128 * VS + n * NF,
                                        [[VS, 128], [1, NF]]),
                            in_=lg[:],
                        )

    nc.compile()
    return nc


def _get_program(t_steps=T):
    key = ("prog", t_steps)
    if key not in _CACHE:
        _CACHE[key] = _build_program(t_steps)
    return _CACHE[key]


def _prep_inputs(inputs):
    bf = ml_dtypes.bfloat16
    ti = np.asarray(inputs["target_input"]).astype(np.int32)      # [B, T]
    enc = np.asarray(inputs["enc_outs"], dtype=np.float32)        # [B, S, H]
    h0 = np.asarray(inputs["h0"], dtype=np.float32)               # [2, B, H]
    c0 = np.asarray(inputs["c0"], dtype=np.float32)
    emb = np.asarray(inputs["emb"], dtype=np.float32)             # [V, E]
    W_ih0 = np.asarray(inputs["W_ih0"], dtype=np.float32)         # [2048, 1024]
    W_hh0 = np.asarray(inputs["W_hh0"], dtype=np.float32)         # [2048, 512]
    W_ih1 = np.asarray(inputs["W_ih1"], dtype=np.float32)
    W_hh1 = np.asarray(inputs["W_hh1"], dtype=np.float32)
    fc_W = np.asarray(inputs["fc_W"], dtype=np.float32)           # [V, 1536]

    def u16(x):
        return np.ascontiguousarray(x.astype(bf)).view(np.uint16)

    idx = np.ascontiguousarray(ti.T.reshape(16, 128))              # t-major
    embt = u16(emb)
    encT = u16(enc.transpose(2, 0, 1).reshape(4, 128, BT))
    encF = u16(enc.reshape(16, 128, H))
    w0 = np.concatenate([W_ih0, W_hh0], axis=1)                    # [2048, 1536]
    w0t = u16(w0.T.reshape(12, 128, 2048))
    w1 = np.concatenate([W_ih1, W_hh1], axis=1)                    # [2048, 1024]
    w1t = u16(w1.T.reshape(8, 128, 2048))
    h0t = np.ascontiguousarray(h0.transpose(0, 2, 1).reshape(2, 4, 128, 32))
    c0t = np.ascontiguousarray(c0.transpose(0, 2, 1).reshape(2, 4, 128, 32))

    common = dict(idx=idx, embt=embt, encT=encT, encF=encF,
                  w0t=w0t, w1t=w1t, h0t=h0t, c0t=c0t)
    in_maps = []
    for core in range(NCORES):
        sl = fc_W[core * VS:(core + 1) * VS]                       # [VS, 1536]
        fct = u16(sl.T.reshape(12, 128, VS))
        m = dict(common)
        m["fct"] = fct
        in_maps.append(m)
    return in_maps


def run_on_hw(inputs, t_steps=T, trace=False):
    from concourse.bass_utils import run_bass_kernel_spmd
    nc = _get_program(t_steps)
    in_maps = _prep_inputs(inputs)
    res = run_bass_kernel_spmd(nc, in_maps, list(range(NCORES)), trace=trace)
    r0 = res.results[0]
    logits = np.concatenate(
        [res.results[i]["logits"].reshape(T, B, VS) for i in range(NCORES)],
        axis=2,
    ).transpose(1, 0, 2)                                           # [B, T, V]
    attn = r0["attn_o"].transpose(1, 0, 2)                         # [B, T, S]
    h = r0["h_o"].transpose(0, 3, 1, 2).reshape(2, 32, 512)
    c = r0["c_o"].transpose(0, 3, 1, 2).reshape(2, 32, 512)
    return (logits, h, c, attn), res


def kernel(**inputs):
    (logits, h, c, attn), _ = run_on_hw(inputs)
    return logits, h, c, attn
